# revision 33
# baseline (speedup 1.0000x reference)
"""Trainium2 Bass kernel for nn_AttentionModule_50002009260608.

B=16, C=512, H=W=24 (HW=576), TF=512, NH=8, CPH=64.
Data-parallel over batch: 2 batch elements per core x 8 cores.
Weights replicated, pre-transposed to [c_in, c_out] bf16 on host; the two
tiny text matvecs (t_m, Wm2 @ t) and the final Wr_b bias add are host-side.

All matmuls bf16 (FWL weight loads). Score matmuls write N=512 + N=64
column chunks into one 2-bank PSUM tile so each exp is a single wide
[sz, 576] ACTIVATE. Softmax denominators come from the ones-block rows of
the AV accumulation (rows 64:128 of pav = column sums broadcast across 64
partitions already), inverted in place with reciprocal_approx_fast - no
exact reciprocal, no partition_broadcast, no cross-softmax normalization
(the cross column's sum rides along in the same reciprocal).
"""

import ml_dtypes
import numpy as np
from contextlib import ExitStack

import concourse.bacc as bacc
import concourse.bass as bass
import concourse.tile as tile
import concourse.mybir as mybir
from concourse import masks
from concourse.bass_utils import run_bass_kernel_spmd

B, C, HW, TF, NH, CPH = 16, 512, 576, 512, 8, 64
NCORES, BPC = 8, B // 8
SCALE = 1.0 / 8.0  # 1/sqrt(CPH)
F32 = mybir.dt.float32
BF16 = mybir.dt.bfloat16
AF = mybir.ActivationFunctionType
OP = mybir.AluOpType
PD = 128
NCC = C // PD                                    # 4 channel chunks
MT = [(0, 128), (128, 128), (256, 128), (384, 128), (512, 64)]  # hw m-tiles
# 128-wide m-tiles: 128-col weight loads qualify for FWL (4-XBUS fast load)
NHALF = [(0, 288), (288, 288)]                   # conv n-halves (1-bank psum)
SCH = [(0, 512), (512, 64)]                      # score col chunks (2-bank psum)
AVCH = [(0, 512), (512, 66)]                     # AV rhs chunks over es cols
CPS = 128                                        # per-head V'T stride (64 V + 64 ones)
TMP = 8                                          # t_m_blk cols (one per head)
ESW = HW + 2                                     # es cols: 576 + cross col + pad


def _body(ctx: ExitStack, tc, d):
    """d: DRAM APs: x[2,512,576](bf16), t_m_blk[2,512,8](bf16),
    tvec[2,512,1](f32), WqT/WkT/WvT/Wm1T/WrT [512,512](bf16, pre-transposed
    [c_in,c_out]), out[2,512,576](bf16)."""
    nc = tc.nc

    wt = ctx.enter_context(tc.tile_pool(name="wt", bufs=1))
    act = ctx.enter_context(tc.tile_pool(name="act", bufs=1))
    expp = ctx.enter_context(tc.tile_pool(name="expp", bufs=1))
    ps = ctx.enter_context(tc.tile_pool(name="ps", bufs=1, space="PSUM"))

    # ---- batch-0 activations first (PE can start within ~2us), then weights,
    # all split per channel-chunk so the first conv group's deps arrive early ----
    xbts = []
    for b in range(BPC):
        xbt = act.tile([PD, NCC * HW], BF16, name=f"xb{b}", tag="xb", bufs=2)
        if b == 0:
            # per-chunk DMAs so the first conv's cc=0 matmul can start as
            # soon as the first 128 channels land
            for cc in range(NCC):
                nc.sync.dma_start(xbt[:, cc * HW:(cc + 1) * HW],
                                  d["x"][b][cc * PD:(cc + 1) * PD])
        xbts.append(xbt)
    W = {}
    for wn in ("WqT", "WkT", "Wm1T", "WvT", "WrT"):
        wtile = wt.tile([PD, NCC * C], BF16, name=f"{wn}_t")
        if wn == "WqT":
            for cc in range(NCC):
                nc.scalar.dma_start(wtile[:, cc * C:(cc + 1) * C],
                                    d[wn][cc * PD:(cc + 1) * PD])
        else:
            nc.scalar.dma_start(wtile[:].rearrange("p (cc o) -> p cc o", cc=NCC),
                                d[wn].rearrange("(cc p) o -> p cc o", p=PD))
        W[wn] = [wtile[:, j * C:(j + 1) * C] for j in range(NCC)]
    onesb = wt.tile([PD, (CPS - CPH) * NH], F32, name="onesb")
    nc.vector.memset(onesb[:], 1.0)
    # persistent V'T tiles, one set per batch: [hw_tile, 8*128]; per head
    # block cols 0:64 = 1.0 (fused softmax column sums -> pav partitions 0:64,
    # where reciprocal_approx_fast needs base_partition 0), cols 64:128 = V_h^T.
    VT = [[wt.tile([sz, NH * CPS], BF16, name=f"vt{b}_{mi}")
           for mi, (m0, sz) in enumerate(MT)] for b in range(BPC)]
    for b in range(BPC):
        for mi, (m0, sz) in enumerate(MT):
            nc.vector.tensor_copy(
                VT[b][mi][:].rearrange("p (h c) -> p h c", h=NH)[:, :, 0:CPH],
                onesb[0:sz, :])

    def conv(name, b, Wn, rhs, outs, evict):
        # outs[ot][:, n] = sum_cc Wn[cc][:, ot*128:+128].T @ rhs[cc][:, n]
        for ot in range(NCC):
            for (n0, nsz) in NHALF:
                p = ps.tile([PD, nsz], F32, tag="conv", bufs=2,
                            name=f"p_{name}{b}_{ot}_{n0}")
                for cc in range(NCC):
                    nc.tensor.matmul(
                        p[:], Wn[cc][:, ot * PD:(ot + 1) * PD],
                        rhs[cc][:, n0:n0 + nsz],
                        start=(cc == 0), stop=(cc == NCC - 1))
                evict(outs[ot][:, n0:n0 + nsz], p[:], ot)

    st8 = {}

    def emit_loads(b):
        xbt = xbts[b]
        if b > 0:
            nc.sync.dma_start(xbt[:].rearrange("p (cc n) -> p cc n", cc=NCC),
                              d["x"][b].rearrange("(cc p) n -> p cc n", p=PD))
        xb = [xbt[:, j * HW:(j + 1) * HW] for j in range(NCC)]
        tvt = act.tile([PD, NCC], F32, name=f"tv{b}", tag="tv", bufs=2)
        nc.sync.dma_start(tvt[:],
                          d["tvec"][b].rearrange("(cc p) one -> p (cc one)", p=PD))
        # host-precomputed softmaxed cross maps, transposed, with a ones col
        crossT = [act.tile([sz, NH + 1], BF16, name=f"crossT{b}_{mi}",
                           tag=f"crossT{mi}", bufs=2)
                  for mi, (m0, sz) in enumerate(MT)]
        for mi, (m0, sz) in enumerate(MT):
            nc.sync.dma_start(crossT[mi][:], d["crossT"][b, m0:m0 + sz, :])
        st8[b] = {"xb": xb, "tvt": tvt, "crossT": crossT}

    def emit_q(b):
        s = st8[b]
        s["Q"] = [act.tile([PD, HW], BF16, name=f"q{b}_{j}", tag=f"q{j}", bufs=2)
                  for j in range(NCC)]
        conv("q", b, W["WqT"], s["xb"], s["Q"],
             lambda dst, p, ot: nc.vector.tensor_copy(dst, p))

    def emit_k(b):
        s = st8[b]
        s["K"] = [act.tile([PD, HW], BF16, name=f"k{b}_{j}", tag=f"k{j}", bufs=2)
                  for j in range(NCC)]
        conv("k", b, W["WkT"], s["xb"], s["K"],
             lambda dst, p, ot: nc.vector.tensor_copy(dst, p))

    def emit_vl_vt(b):
        s = st8[b]
        tvt = s["tvt"]
        vl = [act.tile([PD, HW], BF16, name=f"vl{b}_{j}", tag=f"vl{j}", bufs=2)
              for j in range(NCC)]
        conv("vl", b, W["Wm1T"], s["xb"], vl,
             lambda dst, p, ot: nc.scalar.activation(
                 dst, p, AF.Identity, bias=tvt[:, ot:ot + 1]))
        for mi, (m0, sz) in enumerate(MT):
            p = ps.tile([sz, C], F32, tag="conv", bufs=2, name=f"p_vt{b}_{mi}")
            for cc in range(NCC):
                nc.tensor.matmul(p[:], vl[cc][:, m0:m0 + sz], W["WvT"][cc][:],
                                 start=(cc == 0), stop=(cc == NCC - 1))
            vsrc = p[:].rearrange("p (h c) -> p h c", h=NH)
            vv = VT[b][mi][:].rearrange("p (h c) -> p h c", h=NH)
            nc.vector.tensor_copy(vv[:, :, CPH:CPS], vsrc)

    def emit_cross(b):
        s = st8[b]
        s["outall"] = [act.tile([PD, HW], BF16, name=f"oa{b}_{j}", tag=f"oa{j}",
                                bufs=2) for j in range(NCC)]

    def emit_pair(b, hp):
        s = st8[b]
        K, Q, crossT, outall = s["K"], s["Q"], s["crossT"], s["outall"]
        h2 = (2 * hp, 2 * hp + 1)
        es = [[expp.tile([sz, ESW], BF16, name=f"es{b}_{hp}_{sub}_{mi}",
                         tag=f"es{sub}_{mi}", bufs=2)
               for mi, (m0, sz) in enumerate(MT)] for sub in range(2)]
        for mi, (m0, sz) in enumerate(MT):
            # per-sub 2-bank score tiles; emit each sub's chunk pair together
            # so consecutive score MMs alternate PE quadrants and the next
            # sub's LDWEIGHTS pulls ahead of the in-flight MM.
            for sub in range(2):
                rr = sub * CPH
                pp = ps.tile([sz, 1024], F32, tag="s", bufs=2,
                             name=f"p_s{b}_{hp}_{sub}_{mi}")
                for (n0, nsz) in SCH:
                    nc.tensor.matmul(
                        pp[:, n0:n0 + nsz], K[hp][rr:rr + CPH, m0:m0 + sz],
                        Q[hp][rr:rr + CPH, n0:n0 + nsz],
                        start=True, stop=True, tile_position=(rr, 0),
                        skip_group_check=True)
                nc.scalar.activation(es[sub][mi][:, 0:HW], pp[:, 0:HW],
                                     AF.Exp, scale=SCALE)
                nc.gpsimd.tensor_copy(
                    es[sub][mi][:, HW:ESW],
                    crossT[mi][0:sz, h2[sub]:h2[sub] + 2])
        for sub in range(2):
            h = h2[sub]
            rr = sub * CPH
            pav0 = ps.tile([PD, 512], F32, tag="av0", bufs=1,
                           name=f"p_av0_{b}_{h}")
            pav1 = ps.tile([PD, 66], F32, tag="av1", bufs=1,
                           name=f"p_av1_{b}_{h}")
            pav = (pav0, pav1)
            for mi, (m0, sz) in enumerate(MT):
                lhs = VT[b][mi][:, h * CPS:(h + 1) * CPS]
                st, sp = (mi == 0), (mi == len(MT) - 1)
                for ci, (c0, nsz) in enumerate(AVCH):
                    nc.tensor.matmul(pav[ci][:], lhs,
                                     es[sub][mi][:, c0:c0 + nsz],
                                     start=st, stop=sp)
            # rows 0:64 of pav are the softmax column sums already broadcast
            # across 64 partitions (ones-block); invert in one shot.
            rep = act.tile([CPH, ESW], F32, name=f"rep{b}_{h}", tag="rep",
                           bufs=2)
            nc.vector.reciprocal_approx_fast(rep[:, 0:512], pav0[0:CPH, :])
            nc.vector.reciprocal_approx_fast(rep[:, 512:ESW], pav1[0:CPH, :])
            # cross contribution: V @ cross_h (host pre-softmaxed, sum = 1)
            dst = outall[hp][rr:rr + CPH, :]
            nc.vector.tensor_tensor(dst[:, 0:512], pav0[CPH:2 * CPH, :],
                                    rep[:, 0:512], OP.mult)
            nc.vector.tensor_tensor(dst[:, 512:HW], pav1[CPH:2 * CPH, 0:64],
                                    rep[:, 512:HW], OP.mult)
            nc.vector.tensor_scalar_add(dst, dst, pav1[CPH:2 * CPH, 64:65])

    def emit_final(b, ots=range(NCC)):
        s = st8[b]
        if "fin" not in s:
            s["fin"] = [act.tile([PD, HW], BF16, name=f"fin{b}_{j}",
                                 tag=f"fin{j}", bufs=2) for j in range(NCC)]
        fin = s["fin"]
        for ot in ots:
            for hi, (n0, nsz) in enumerate(NHALF):
                p = ps.tile([PD, nsz], F32, tag="conv", bufs=2,
                            name=f"p_fin{b}_{ot}_{n0}")
                for cc in range(NCC):
                    nc.tensor.matmul(
                        p[:], W["WrT"][cc][:, ot * PD:(ot + 1) * PD],
                        s["outall"][cc][:, n0:n0 + nsz],
                        start=(cc == 0), stop=(cc == NCC - 1))
                if hi == 0:
                    nc.vector.tensor_copy(fin[ot][:, n0:n0 + nsz], p[:])
                else:
                    nc.scalar.copy(fin[ot][:, n0:n0 + nsz], p[:])
            nc.sync.dma_start(d["out"][b, ot * PD:(ot + 1) * PD, :], fin[ot][:])

    # interleave batch 1's PE-dense conv work into batch 0's head phase so the
    # tensor engine stays busy (and the HAM clock stays warm) throughout.
    emit_loads(0)
    emit_q(0)
    emit_k(0)
    emit_vl_vt(0)
    emit_cross(0)
    emit_loads(1)
    emit_pair(0, 0)
    emit_q(1)
    emit_pair(0, 1)
    emit_k(1)
    emit_pair(0, 2)
    emit_vl_vt(1)
    emit_pair(0, 3)
    emit_cross(1)
    emit_pair(1, 0)
    emit_final(0, [0])
    emit_pair(1, 1)
    emit_final(0, [1])
    emit_pair(1, 2)
    emit_final(0, [2])
    emit_pair(1, 3)
    emit_final(0, [3])
    emit_final(1)


_CACHE = {}


def _build():
    if "nc" in _CACHE:
        return _CACHE["nc"], _CACHE["out"]
    nc = bacc.Bacc("TRN2", target_bir_lowering=False, debug=False,
                   num_devices=NCORES)
    d = {
        "x": nc.dram_tensor("x", [BPC, C, HW], BF16, kind="ExternalInput").ap(),
        "crossT": nc.dram_tensor("crossT", [BPC, HW, NH + 1], BF16,
                                 kind="ExternalInput").ap(),
        "tvec": nc.dram_tensor("tvec", [BPC, C, 1], F32, kind="ExternalInput").ap(),
        "out": nc.dram_tensor("out", [BPC, C, HW], BF16,
                              kind="ExternalOutput").ap(),
    }
    for wn in ("WqT", "WkT", "WvT", "Wm1T", "WrT"):
        d[wn] = nc.dram_tensor(wn, [C, C], BF16, kind="ExternalInput").ap()
    with tile.TileContext(nc) as tc:
        with ExitStack() as ctx:
            _body(ctx, tc, d)
    nc.compile()
    _CACHE["nc"], _CACHE["out"] = nc, d["out"].tensor.name
    return nc, _CACHE["out"]


def _prep_inputs(x, t, Wk, Wq, Wt_w, Wt_b, Wm, Wv, Wr_w, Wr_b):
    f = np.float32
    x = np.asarray(x, f).reshape(B, C, HW)
    t = np.asarray(t, f)
    t_m = (t @ np.asarray(Wt_w, f).T + np.asarray(Wt_b, f)).reshape(B, NH, CPH)
    # cross attention softmax on host: [B, NH, HW]
    logits = np.einsum("bhcm,bhc->bhm", x.reshape(B, NH, CPH, HW), t_m) * SCALE
    logits -= logits.max(axis=-1, keepdims=True)
    ce = np.exp(logits)
    cross = ce / ce.sum(axis=-1, keepdims=True)
    crossT = np.ones((B, HW, NH + 1), f)
    crossT[:, :, 0:NH] = cross.transpose(0, 2, 1)
    tvec = (t @ np.asarray(Wm, f)[:, C:].T).reshape(B, C, 1)
    bf = ml_dtypes.bfloat16
    com = {
        "WqT": np.ascontiguousarray(np.asarray(Wq, f).T).astype(bf),
        "WkT": np.ascontiguousarray(np.asarray(Wk, f).T).astype(bf),
        "WvT": np.ascontiguousarray(np.asarray(Wv, f).T).astype(bf),
        "Wm1T": np.ascontiguousarray(np.asarray(Wm, f)[:, :C].T).astype(bf),
        "WrT": np.ascontiguousarray(np.asarray(Wr_w, f).T).astype(bf),
    }
    maps = []
    for c in range(NCORES):
        sl = slice(c * BPC, (c + 1) * BPC)
        m = dict(com)
        m["x"] = np.ascontiguousarray(x[sl]).astype(bf)
        m["crossT"] = np.ascontiguousarray(crossT[sl]).astype(bf)
        m["tvec"] = np.ascontiguousarray(tvec[sl])
        maps.append(m)
    return maps


def kernel(x, t, Wk, Wq, Wt_w, Wt_b, Wm, Wv, Wr_w, Wr_b, _trace=False):
    nc, out_name = _build()
    maps = _prep_inputs(x, t, Wk, Wq, Wt_w, Wt_b, Wm, Wv, Wr_w, Wr_b)
    res = run_bass_kernel_spmd(nc, maps, core_ids=list(range(NCORES)),
                               trace=_trace)
    out = np.concatenate([res.results[c][out_name] for c in range(NCORES)],
                         axis=0).astype(np.float32)
    out = out.reshape(B, C, 24, 24) + np.asarray(Wr_b, np.float32).reshape(
        1, C, 1, 1)
    if _trace:
        kernel.last_results = res
    return out


# revision 34
# speedup vs baseline: 1.0103x; 1.0103x over previous
"""Trainium2 Bass kernel for nn_AttentionModule_50002009260608.

B=16, C=512, H=W=24 (HW=576), TF=512, NH=8, CPH=64.
Data-parallel over batch: 2 batch elements per core x 8 cores.
Weights replicated, pre-transposed to [c_in, c_out] bf16 on host; the two
tiny text matvecs (t_m, Wm2 @ t) and the final Wr_b bias add are host-side.

All matmuls bf16 (FWL weight loads). Score matmuls write N=512 + N=64
column chunks into one 2-bank PSUM tile so each exp is a single wide
[sz, 576] ACTIVATE. Softmax denominators come from the ones-block rows of
the AV accumulation (rows 64:128 of pav = column sums broadcast across 64
partitions already), inverted in place with reciprocal_approx_fast - no
exact reciprocal, no partition_broadcast, no cross-softmax normalization
(the cross column's sum rides along in the same reciprocal).
"""

import ml_dtypes
import numpy as np
from contextlib import ExitStack

import concourse.bacc as bacc
import concourse.bass as bass
import concourse.tile as tile
import concourse.mybir as mybir
from concourse import masks
from concourse.bass_utils import run_bass_kernel_spmd

B, C, HW, TF, NH, CPH = 16, 512, 576, 512, 8, 64
NCORES, BPC = 8, B // 8
SCALE = 1.0 / 8.0  # 1/sqrt(CPH)
F32 = mybir.dt.float32
BF16 = mybir.dt.bfloat16
AF = mybir.ActivationFunctionType
OP = mybir.AluOpType
PD = 128
NCC = C // PD                                    # 4 channel chunks
MT = [(0, 128), (128, 128), (256, 128), (384, 128), (512, 64)]  # hw m-tiles
# 128-wide m-tiles: 128-col weight loads qualify for FWL (4-XBUS fast load)
NHALF = [(0, 288), (288, 288)]                   # conv n-halves (1-bank psum)
SCH = [(0, 512), (512, 64)]                      # score col chunks (2-bank psum)
AVCH = [(0, 512), (512, 66)]                     # AV rhs chunks over es cols
CPS = 128                                        # per-head V'T stride (64 V + 64 ones)
TMP = 8                                          # t_m_blk cols (one per head)
ESW = HW + 2                                     # es cols: 576 + cross col + pad


def _body(ctx: ExitStack, tc, d):
    """d: DRAM APs: x[2,512,576](bf16), t_m_blk[2,512,8](bf16),
    tvec[2,512,1](f32), WqT/WkT/WvT/Wm1T/WrT [512,512](bf16, pre-transposed
    [c_in,c_out]), out[2,512,576](bf16)."""
    nc = tc.nc

    wt = ctx.enter_context(tc.tile_pool(name="wt", bufs=1))
    act = ctx.enter_context(tc.tile_pool(name="act", bufs=1))
    expp = ctx.enter_context(tc.tile_pool(name="expp", bufs=1))
    ps = ctx.enter_context(tc.tile_pool(name="ps", bufs=1, space="PSUM"))

    # ---- batch-0 activations first (PE can start within ~2us), then weights,
    # all split per channel-chunk so the first conv group's deps arrive early ----
    xbts = []
    for b in range(BPC):
        xbt = act.tile([PD, NCC * HW], BF16, name=f"xb{b}", tag="xb", bufs=2)
        if b == 0:
            # per-chunk DMAs so the first conv's cc=0 matmul can start as
            # soon as the first 128 channels land
            for cc in range(NCC):
                nc.sync.dma_start(xbt[:, cc * HW:(cc + 1) * HW],
                                  d["x"][b][cc * PD:(cc + 1) * PD])
        xbts.append(xbt)
    W = {}
    for wn in ("WqT", "WkT", "Wm1T", "WvT", "WrT"):
        wtile = wt.tile([PD, NCC * C], BF16, name=f"{wn}_t")
        if wn == "WqT":
            for cc in range(NCC):
                nc.scalar.dma_start(wtile[:, cc * C:(cc + 1) * C],
                                    d[wn][cc * PD:(cc + 1) * PD])
        else:
            nc.scalar.dma_start(wtile[:].rearrange("p (cc o) -> p cc o", cc=NCC),
                                d[wn].rearrange("(cc p) o -> p cc o", p=PD))
        W[wn] = [wtile[:, j * C:(j + 1) * C] for j in range(NCC)]
    onesb = wt.tile([PD, (CPS - CPH) * NH], F32, name="onesb")
    nc.vector.memset(onesb[:], 1.0)
    # persistent V'T tiles, one set per batch: [hw_tile, 8*128]; per head
    # block cols 0:64 = 1.0 (fused softmax column sums -> pav partitions 0:64,
    # where reciprocal_approx_fast needs base_partition 0), cols 64:128 = V_h^T.
    VT = [[wt.tile([sz, NH * CPS], BF16, name=f"vt{b}_{mi}")
           for mi, (m0, sz) in enumerate(MT)] for b in range(BPC)]
    for b in range(BPC):
        for mi, (m0, sz) in enumerate(MT):
            nc.vector.tensor_copy(
                VT[b][mi][:].rearrange("p (h c) -> p h c", h=NH)[:, :, 0:CPH],
                onesb[0:sz, :])

    def conv(name, b, Wn, rhs, outs, evict):
        # outs[ot][:, n] = sum_cc Wn[cc][:, ot*128:+128].T @ rhs[cc][:, n]
        for ot in range(NCC):
            for (n0, nsz) in NHALF:
                p = ps.tile([PD, nsz], F32, tag="conv", bufs=2,
                            name=f"p_{name}{b}_{ot}_{n0}")
                for cc in range(NCC):
                    nc.tensor.matmul(
                        p[:], Wn[cc][:, ot * PD:(ot + 1) * PD],
                        rhs[cc][:, n0:n0 + nsz],
                        start=(cc == 0), stop=(cc == NCC - 1))
                evict(outs[ot][:, n0:n0 + nsz], p[:], ot)

    st8 = {}

    def emit_loads(b):
        xbt = xbts[b]
        if b > 0:
            nc.sync.dma_start(xbt[:].rearrange("p (cc n) -> p cc n", cc=NCC),
                              d["x"][b].rearrange("(cc p) n -> p cc n", p=PD))
        xb = [xbt[:, j * HW:(j + 1) * HW] for j in range(NCC)]
        tvt = act.tile([PD, NCC], F32, name=f"tv{b}", tag="tv", bufs=2)
        nc.sync.dma_start(tvt[:],
                          d["tvec"][b].rearrange("(cc p) one -> p (cc one)", p=PD))
        # host-precomputed softmaxed cross maps, transposed, with a ones col
        crossT = [act.tile([sz, NH + 1], BF16, name=f"crossT{b}_{mi}",
                           tag=f"crossT{mi}", bufs=2)
                  for mi, (m0, sz) in enumerate(MT)]
        for mi, (m0, sz) in enumerate(MT):
            nc.sync.dma_start(crossT[mi][:], d["crossT"][b, m0:m0 + sz, :])
        st8[b] = {"xb": xb, "tvt": tvt, "crossT": crossT}

    def emit_q(b):
        s = st8[b]
        s["Q"] = [act.tile([PD, HW], BF16, name=f"q{b}_{j}", tag=f"q{j}", bufs=2)
                  for j in range(NCC)]
        conv("q", b, W["WqT"], s["xb"], s["Q"],
             lambda dst, p, ot: nc.vector.tensor_copy(dst, p))

    def emit_k(b):
        s = st8[b]
        s["K"] = [act.tile([PD, HW], BF16, name=f"k{b}_{j}", tag=f"k{j}", bufs=2)
                  for j in range(NCC)]
        conv("k", b, W["WkT"], s["xb"], s["K"],
             lambda dst, p, ot: nc.vector.tensor_copy(dst, p))

    def emit_vl_vt(b):
        s = st8[b]
        tvt = s["tvt"]
        vl = [act.tile([PD, HW], BF16, name=f"vl{b}_{j}", tag=f"vl{j}", bufs=2)
              for j in range(NCC)]
        conv("vl", b, W["Wm1T"], s["xb"], vl,
             lambda dst, p, ot: nc.scalar.activation(
                 dst, p, AF.Identity, bias=tvt[:, ot:ot + 1]))
        for mi, (m0, sz) in enumerate(MT):
            p = ps.tile([sz, C], F32, tag="conv", bufs=2, name=f"p_vt{b}_{mi}")
            for cc in range(NCC):
                nc.tensor.matmul(p[:], vl[cc][:, m0:m0 + sz], W["WvT"][cc][:],
                                 start=(cc == 0), stop=(cc == NCC - 1))
            vsrc = p[:].rearrange("p (h c) -> p h c", h=NH)
            vv = VT[b][mi][:].rearrange("p (h c) -> p h c", h=NH)
            nc.vector.tensor_copy(vv[:, :, CPH:CPS], vsrc)

    def emit_cross(b):
        s = st8[b]
        s["outall"] = [act.tile([PD, HW], BF16, name=f"oa{b}_{j}", tag=f"oa{j}",
                                bufs=2) for j in range(NCC)]

    def emit_pair(b, hp):
        s = st8[b]
        K, Q, crossT, outall = s["K"], s["Q"], s["crossT"], s["outall"]
        h2 = (2 * hp, 2 * hp + 1)
        es = [[expp.tile([sz, ESW], BF16, name=f"es{b}_{hp}_{sub}_{mi}",
                         tag=f"es{sub}_{mi}", bufs=2)
               for mi, (m0, sz) in enumerate(MT)] for sub in range(2)]
        for mi, (m0, sz) in enumerate(MT):
            # per-sub 2-bank score tiles; emit each sub's chunk pair together
            # so consecutive score MMs alternate PE quadrants and the next
            # sub's LDWEIGHTS pulls ahead of the in-flight MM.
            for sub in range(2):
                rr = sub * CPH
                pp = ps.tile([sz, 1024], F32, tag="s", bufs=2,
                             name=f"p_s{b}_{hp}_{sub}_{mi}")
                for (n0, nsz) in SCH:
                    nc.tensor.matmul(
                        pp[:, n0:n0 + nsz], K[hp][rr:rr + CPH, m0:m0 + sz],
                        Q[hp][rr:rr + CPH, n0:n0 + nsz],
                        start=True, stop=True, tile_position=(rr, 0),
                        skip_group_check=True)
                nc.scalar.activation(es[sub][mi][:, 0:HW], pp[:, 0:HW],
                                     AF.Exp, scale=SCALE)
                nc.gpsimd.tensor_copy(
                    es[sub][mi][:, HW:ESW],
                    crossT[mi][0:sz, h2[sub]:h2[sub] + 2])
        for sub in range(2):
            h = h2[sub]
            rr = sub * CPH
            pav0 = ps.tile([PD, 512], F32, tag="av0", bufs=1,
                           name=f"p_av0_{b}_{h}")
            pav1 = ps.tile([PD, 66], F32, tag="av1", bufs=1,
                           name=f"p_av1_{b}_{h}")
            pav = (pav0, pav1)
            for mi, (m0, sz) in enumerate(MT):
                lhs = VT[b][mi][:, h * CPS:(h + 1) * CPS]
                st, sp = (mi == 0), (mi == len(MT) - 1)
                for ci, (c0, nsz) in enumerate(AVCH):
                    nc.tensor.matmul(pav[ci][:], lhs,
                                     es[sub][mi][:, c0:c0 + nsz],
                                     start=st, stop=sp)
            # rows 0:64 of pav are the softmax column sums already broadcast
            # across 64 partitions (ones-block); invert in one shot.
            rep = act.tile([CPH, ESW], F32, name=f"rep{b}_{h}", tag="rep",
                           bufs=2)
            nc.vector.reciprocal_approx_fast(rep[:, 0:512], pav0[0:CPH, :])
            nc.vector.reciprocal_approx_fast(rep[:, 512:ESW], pav1[0:CPH, :])
            # cross contribution: V @ cross_h (host pre-softmaxed, sum = 1)
            dst = outall[hp][rr:rr + CPH, :]
            nc.vector.tensor_tensor(dst[:, 0:512], pav0[CPH:2 * CPH, :],
                                    rep[:, 0:512], OP.mult)
            nc.vector.tensor_tensor(dst[:, 512:HW], pav1[CPH:2 * CPH, 0:64],
                                    rep[:, 512:HW], OP.mult)
            nc.vector.tensor_scalar_add(dst, dst, pav1[CPH:2 * CPH, 64:65])

    def emit_final(b, ots=range(NCC)):
        s = st8[b]
        if "fin" not in s:
            s["fin"] = [act.tile([PD, HW], BF16, name=f"fin{b}_{j}",
                                 tag=f"fin{j}", bufs=2) for j in range(NCC)]
        fin = s["fin"]
        for ot in ots:
            for hi, (n0, nsz) in enumerate(NHALF):
                p = ps.tile([PD, nsz], F32, tag="conv", bufs=2,
                            name=f"p_fin{b}_{ot}_{n0}")
                for cc in range(NCC):
                    nc.tensor.matmul(
                        p[:], W["WrT"][cc][:, ot * PD:(ot + 1) * PD],
                        s["outall"][cc][:, n0:n0 + nsz],
                        start=(cc == 0), stop=(cc == NCC - 1))
                if hi == 0:
                    nc.vector.tensor_copy(fin[ot][:, n0:n0 + nsz], p[:])
                else:
                    nc.scalar.copy(fin[ot][:, n0:n0 + nsz], p[:])
            nc.sync.dma_start(d["out"][b, ot * PD:(ot + 1) * PD, :], fin[ot][:])

    # interleave batch 1's PE-dense conv work into batch 0's head phase so the
    # tensor engine stays busy (and the HAM clock stays warm) throughout.
    emit_loads(0)
    emit_q(0)
    emit_loads(1)
    emit_k(0)
    emit_vl_vt(0)
    emit_cross(0)
    emit_pair(0, 0)
    emit_q(1)
    emit_pair(0, 1)
    emit_k(1)
    emit_pair(0, 2)
    emit_pair(0, 3)
    emit_vl_vt(1)
    emit_cross(1)
    emit_pair(1, 0)
    emit_final(0, [0])
    emit_pair(1, 1)
    emit_final(0, [1])
    emit_pair(1, 2)
    emit_final(0, [2])
    emit_pair(1, 3)
    emit_final(0, [3])
    emit_final(1)


_CACHE = {}


def _build():
    if "nc" in _CACHE:
        return _CACHE["nc"], _CACHE["out"]
    nc = bacc.Bacc("TRN2", target_bir_lowering=False, debug=False,
                   num_devices=NCORES)
    d = {
        "x": nc.dram_tensor("x", [BPC, C, HW], BF16, kind="ExternalInput").ap(),
        "crossT": nc.dram_tensor("crossT", [BPC, HW, NH + 1], BF16,
                                 kind="ExternalInput").ap(),
        "tvec": nc.dram_tensor("tvec", [BPC, C, 1], F32, kind="ExternalInput").ap(),
        "out": nc.dram_tensor("out", [BPC, C, HW], BF16,
                              kind="ExternalOutput").ap(),
    }
    for wn in ("WqT", "WkT", "WvT", "Wm1T", "WrT"):
        d[wn] = nc.dram_tensor(wn, [C, C], BF16, kind="ExternalInput").ap()
    with tile.TileContext(nc) as tc:
        with ExitStack() as ctx:
            _body(ctx, tc, d)
    nc.compile()
    _CACHE["nc"], _CACHE["out"] = nc, d["out"].tensor.name
    return nc, _CACHE["out"]


def _prep_inputs(x, t, Wk, Wq, Wt_w, Wt_b, Wm, Wv, Wr_w, Wr_b):
    f = np.float32
    x = np.asarray(x, f).reshape(B, C, HW)
    t = np.asarray(t, f)
    t_m = (t @ np.asarray(Wt_w, f).T + np.asarray(Wt_b, f)).reshape(B, NH, CPH)
    # cross attention softmax on host: [B, NH, HW]
    logits = np.einsum("bhcm,bhc->bhm", x.reshape(B, NH, CPH, HW), t_m) * SCALE
    logits -= logits.max(axis=-1, keepdims=True)
    ce = np.exp(logits)
    cross = ce / ce.sum(axis=-1, keepdims=True)
    crossT = np.ones((B, HW, NH + 1), f)
    crossT[:, :, 0:NH] = cross.transpose(0, 2, 1)
    tvec = (t @ np.asarray(Wm, f)[:, C:].T).reshape(B, C, 1)
    bf = ml_dtypes.bfloat16
    com = {
        "WqT": np.ascontiguousarray(np.asarray(Wq, f).T).astype(bf),
        "WkT": np.ascontiguousarray(np.asarray(Wk, f).T).astype(bf),
        "WvT": np.ascontiguousarray(np.asarray(Wv, f).T).astype(bf),
        "Wm1T": np.ascontiguousarray(np.asarray(Wm, f)[:, :C].T).astype(bf),
        "WrT": np.ascontiguousarray(np.asarray(Wr_w, f).T).astype(bf),
    }
    maps = []
    for c in range(NCORES):
        sl = slice(c * BPC, (c + 1) * BPC)
        m = dict(com)
        m["x"] = np.ascontiguousarray(x[sl]).astype(bf)
        m["crossT"] = np.ascontiguousarray(crossT[sl]).astype(bf)
        m["tvec"] = np.ascontiguousarray(tvec[sl])
        maps.append(m)
    return maps


def kernel(x, t, Wk, Wq, Wt_w, Wt_b, Wm, Wv, Wr_w, Wr_b, _trace=False):
    nc, out_name = _build()
    maps = _prep_inputs(x, t, Wk, Wq, Wt_w, Wt_b, Wm, Wv, Wr_w, Wr_b)
    res = run_bass_kernel_spmd(nc, maps, core_ids=list(range(NCORES)),
                               trace=_trace)
    out = np.concatenate([res.results[c][out_name] for c in range(NCORES)],
                         axis=0).astype(np.float32)
    out = out.reshape(B, C, 24, 24) + np.asarray(Wr_b, np.float32).reshape(
        1, C, 1, 1)
    if _trace:
        kernel.last_results = res
    return out


# revision 35
# speedup vs baseline: 1.0172x; 1.0069x over previous
"""Trainium2 Bass kernel for nn_AttentionModule_50002009260608.

B=16, C=512, H=W=24 (HW=576), TF=512, NH=8, CPH=64.
Data-parallel over batch: 2 batch elements per core x 8 cores.
Weights replicated, pre-transposed to [c_in, c_out] bf16 on host; the two
tiny text matvecs (t_m, Wm2 @ t) and the final Wr_b bias add are host-side.

All matmuls bf16 (FWL weight loads). Score matmuls write N=512 + N=64
column chunks into one 2-bank PSUM tile so each exp is a single wide
[sz, 576] ACTIVATE. Softmax denominators come from the ones-block rows of
the AV accumulation (rows 64:128 of pav = column sums broadcast across 64
partitions already), inverted in place with reciprocal_approx_fast - no
exact reciprocal, no partition_broadcast, no cross-softmax normalization
(the cross column's sum rides along in the same reciprocal).
"""

import ml_dtypes
import numpy as np
from contextlib import ExitStack

import concourse.bacc as bacc
import concourse.bass as bass
import concourse.tile as tile
import concourse.mybir as mybir
from concourse import masks
from concourse.bass_utils import run_bass_kernel_spmd

B, C, HW, TF, NH, CPH = 16, 512, 576, 512, 8, 64
NCORES, BPC = 8, B // 8
SCALE = 1.0 / 8.0  # 1/sqrt(CPH)
F32 = mybir.dt.float32
BF16 = mybir.dt.bfloat16
AF = mybir.ActivationFunctionType
OP = mybir.AluOpType
PD = 128
NCC = C // PD                                    # 4 channel chunks
MT = [(0, 128), (128, 128), (256, 128), (384, 128), (512, 64)]  # hw m-tiles
# 128-wide m-tiles: 128-col weight loads qualify for FWL (4-XBUS fast load)
NHALF = [(0, 288), (288, 288)]                   # conv n-halves (1-bank psum)
SCH = [(0, 512), (512, 64)]                      # score col chunks (2-bank psum)
AVCH = [(0, 512), (512, 66)]                     # AV rhs chunks over es cols
CPS = 128                                        # per-head V'T stride (64 V + 64 ones)
TMP = 8                                          # t_m_blk cols (one per head)
ESW = HW + 2                                     # es cols: 576 + cross col + pad


def _body(ctx: ExitStack, tc, d):
    """d: DRAM APs: x[2,512,576](bf16), t_m_blk[2,512,8](bf16),
    tvec[2,512,1](f32), WqT/WkT/WvT/Wm1T/WrT [512,512](bf16, pre-transposed
    [c_in,c_out]), out[2,512,576](bf16)."""
    nc = tc.nc

    wt = ctx.enter_context(tc.tile_pool(name="wt", bufs=1))
    act = ctx.enter_context(tc.tile_pool(name="act", bufs=1))
    expp = ctx.enter_context(tc.tile_pool(name="expp", bufs=1))
    ps = ctx.enter_context(tc.tile_pool(name="ps", bufs=1, space="PSUM"))

    # ---- batch-0 activations first (PE can start within ~2us), then weights,
    # all split per channel-chunk so the first conv group's deps arrive early ----
    xbts = []
    for b in range(BPC):
        xbt = act.tile([PD, NCC * HW], BF16, name=f"xb{b}", tag="xb", bufs=2)
        if b == 0:
            # per-chunk DMAs so the first conv's cc=0 matmul can start as
            # soon as the first 128 channels land
            for cc in range(NCC):
                nc.sync.dma_start(xbt[:, cc * HW:(cc + 1) * HW],
                                  d["x"][b][cc * PD:(cc + 1) * PD])
        xbts.append(xbt)
    W = {}
    for wn in ("WqT", "WkT", "Wm1T", "WvT", "WrT"):
        wtile = wt.tile([PD, NCC * C], BF16, name=f"{wn}_t")
        if wn == "WqT":
            for cc in range(NCC):
                nc.scalar.dma_start(wtile[:, cc * C:(cc + 1) * C],
                                    d[wn][cc * PD:(cc + 1) * PD])
        else:
            nc.scalar.dma_start(wtile[:].rearrange("p (cc o) -> p cc o", cc=NCC),
                                d[wn].rearrange("(cc p) o -> p cc o", p=PD))
        W[wn] = [wtile[:, j * C:(j + 1) * C] for j in range(NCC)]
    onesb = wt.tile([PD, (CPS - CPH) * NH], F32, name="onesb")
    nc.vector.memset(onesb[:], 1.0)
    # persistent V'T tiles, one set per batch: [hw_tile, 8*128]; per head
    # block cols 0:64 = 1.0 (fused softmax column sums -> pav partitions 0:64,
    # where reciprocal_approx_fast needs base_partition 0), cols 64:128 = V_h^T.
    VT = [[wt.tile([sz, NH * CPS], BF16, name=f"vt{b}_{mi}")
           for mi, (m0, sz) in enumerate(MT)] for b in range(BPC)]
    for b in range(BPC):
        for mi, (m0, sz) in enumerate(MT):
            nc.vector.tensor_copy(
                VT[b][mi][:].rearrange("p (h c) -> p h c", h=NH)[:, :, 0:CPH],
                onesb[0:sz, :])

    def conv(name, b, Wn, rhs, outs, evict):
        # outs[ot][:, n] = sum_cc Wn[cc][:, ot*128:+128].T @ rhs[cc][:, n]
        for ot in range(NCC):
            for (n0, nsz) in NHALF:
                p = ps.tile([PD, nsz], F32, tag="conv", bufs=2,
                            name=f"p_{name}{b}_{ot}_{n0}")
                for cc in range(NCC):
                    nc.tensor.matmul(
                        p[:], Wn[cc][:, ot * PD:(ot + 1) * PD],
                        rhs[cc][:, n0:n0 + nsz],
                        start=(cc == 0), stop=(cc == NCC - 1))
                evict(outs[ot][:, n0:n0 + nsz], p[:], ot)

    st8 = {}

    def emit_loads(b):
        xbt = xbts[b]
        if b > 0:
            nc.sync.dma_start(xbt[:].rearrange("p (cc n) -> p cc n", cc=NCC),
                              d["x"][b].rearrange("(cc p) n -> p cc n", p=PD))
        xb = [xbt[:, j * HW:(j + 1) * HW] for j in range(NCC)]
        tvt = act.tile([PD, NCC], F32, name=f"tv{b}", tag="tv", bufs=2)
        nc.sync.dma_start(tvt[:],
                          d["tvec"][b].rearrange("(cc p) one -> p (cc one)", p=PD))
        # host-precomputed softmaxed cross maps, transposed, with a ones col
        crossT = [act.tile([sz, NH + 1], BF16, name=f"crossT{b}_{mi}",
                           tag=f"crossT{mi}", bufs=2)
                  for mi, (m0, sz) in enumerate(MT)]
        for mi, (m0, sz) in enumerate(MT):
            nc.sync.dma_start(crossT[mi][:], d["crossT"][b, m0:m0 + sz, :])
        st8[b] = {"xb": xb, "tvt": tvt, "crossT": crossT}

    def emit_q(b):
        s = st8[b]
        s["Q"] = [act.tile([PD, HW], BF16, name=f"q{b}_{j}", tag=f"q{j}", bufs=2)
                  for j in range(NCC)]
        conv("q", b, W["WqT"], s["xb"], s["Q"],
             lambda dst, p, ot: nc.vector.tensor_copy(dst, p))

    def emit_k(b):
        s = st8[b]
        s["K"] = [act.tile([PD, HW], BF16, name=f"k{b}_{j}", tag=f"k{j}", bufs=2)
                  for j in range(NCC)]
        conv("k", b, W["WkT"], s["xb"], s["K"],
             lambda dst, p, ot: nc.vector.tensor_copy(dst, p))

    def emit_vl_vt(b):
        s = st8[b]
        tvt = s["tvt"]
        vl = [act.tile([PD, HW], BF16, name=f"vl{b}_{j}", tag=f"vl{j}", bufs=2)
              for j in range(NCC)]
        conv("vl", b, W["Wm1T"], s["xb"], vl,
             lambda dst, p, ot: nc.scalar.activation(
                 dst, p, AF.Identity, bias=tvt[:, ot:ot + 1]))
        for mi, (m0, sz) in enumerate(MT):
            p = ps.tile([sz, C], F32, tag="conv", bufs=2, name=f"p_vt{b}_{mi}")
            for cc in range(NCC):
                nc.tensor.matmul(p[:], vl[cc][:, m0:m0 + sz], W["WvT"][cc][:],
                                 start=(cc == 0), stop=(cc == NCC - 1))
            vsrc = p[:].rearrange("p (h c) -> p h c", h=NH)
            vv = VT[b][mi][:].rearrange("p (h c) -> p h c", h=NH)
            nc.vector.tensor_copy(vv[:, :, CPH:CPS], vsrc)

    def emit_cross(b):
        s = st8[b]
        s["outall"] = [act.tile([PD, HW], BF16, name=f"oa{b}_{j}", tag=f"oa{j}",
                                bufs=2) for j in range(NCC)]

    def emit_pair(b, hp, heartbeat=False):
        s = st8[b]
        K, Q, crossT, outall = s["K"], s["Q"], s["crossT"], s["outall"]
        h2 = (2 * hp, 2 * hp + 1)
        es = [[expp.tile([sz, ESW], BF16, name=f"es{b}_{hp}_{sub}_{mi}",
                         tag=f"es{sub}_{mi}", bufs=2)
               for mi, (m0, sz) in enumerate(MT)] for sub in range(2)]
        for mi, (m0, sz) in enumerate(MT):
            # per-sub 2-bank score tiles; emit each sub's chunk pair together
            # so consecutive score MMs alternate PE quadrants and the next
            # sub's LDWEIGHTS pulls ahead of the in-flight MM.
            for sub in range(2):
                rr = sub * CPH
                pp = ps.tile([sz, 1024], F32, tag="s", bufs=2,
                             name=f"p_s{b}_{hp}_{sub}_{mi}")
                for (n0, nsz) in SCH:
                    nc.tensor.matmul(
                        pp[:, n0:n0 + nsz], K[hp][rr:rr + CPH, m0:m0 + sz],
                        Q[hp][rr:rr + CPH, n0:n0 + nsz],
                        start=True, stop=True, tile_position=(rr, 0),
                        skip_group_check=True)
                nc.scalar.activation(es[sub][mi][:, 0:HW], pp[:, 0:HW],
                                     AF.Exp, scale=SCALE)
                nc.gpsimd.tensor_copy(
                    es[sub][mi][:, HW:ESW],
                    crossT[mi][0:sz, h2[sub]:h2[sub] + 2])
        for sub in range(2):
            h = h2[sub]
            rr = sub * CPH
            pav0 = ps.tile([PD, 512], F32, tag="av0", bufs=1,
                           name=f"p_av0_{b}_{h}")
            pav1 = ps.tile([PD, 66], F32, tag="av1", bufs=1,
                           name=f"p_av1_{b}_{h}")
            pav = (pav0, pav1)
            for mi, (m0, sz) in enumerate(MT):
                lhs = VT[b][mi][:, h * CPS:(h + 1) * CPS]
                st, sp = (mi == 0), (mi == len(MT) - 1)
                for ci, (c0, nsz) in enumerate(AVCH):
                    nc.tensor.matmul(pav[ci][:], lhs,
                                     es[sub][mi][:, c0:c0 + nsz],
                                     start=st, stop=sp)
            # rows 0:64 of pav are the softmax column sums already broadcast
            # across 64 partitions (ones-block); invert in one shot.
            rep = act.tile([CPH, ESW], F32, name=f"rep{b}_{h}", tag="rep",
                           bufs=2)
            nc.vector.reciprocal_approx_fast(rep[:, 0:512], pav0[0:CPH, :])
            nc.vector.reciprocal_approx_fast(rep[:, 512:ESW], pav1[0:CPH, :])
            # cross contribution: V @ cross_h (host pre-softmaxed, sum = 1)
            dst = outall[hp][rr:rr + CPH, :]
            nc.vector.tensor_tensor(dst[:, 0:512], pav0[CPH:2 * CPH, :],
                                    rep[:, 0:512], OP.mult)
            nc.vector.tensor_tensor(dst[:, 512:HW], pav1[CPH:2 * CPH, 0:64],
                                    rep[:, 512:HW], OP.mult)
            nc.vector.tensor_scalar_add(dst, dst, pav1[CPH:2 * CPH, 64:65])
            if heartbeat:
                # keep the PE HAM clock warm through this ACT/DVE-bound
                # stretch so the trailing final-conv matmuls start at 2.4GHz
                hb = ps.tile([PD, 64], F32, tag="conv", bufs=2,
                             name=f"hb{b}_{h}")
                nc.tensor.matmul(hb[:], VT[b][0][:, 0:PD],
                                 es[sub][0][:, 0:64], start=True, stop=True)

    def emit_final(b, ots=range(NCC)):
        s = st8[b]
        if "fin" not in s:
            s["fin"] = [act.tile([PD, HW], BF16, name=f"fin{b}_{j}",
                                 tag=f"fin{j}", bufs=2) for j in range(NCC)]
        fin = s["fin"]
        for ot in ots:
            for hi, (n0, nsz) in enumerate(NHALF):
                p = ps.tile([PD, nsz], F32, tag="conv", bufs=2,
                            name=f"p_fin{b}_{ot}_{n0}")
                for cc in range(NCC):
                    nc.tensor.matmul(
                        p[:], W["WrT"][cc][:, ot * PD:(ot + 1) * PD],
                        s["outall"][cc][:, n0:n0 + nsz],
                        start=(cc == 0), stop=(cc == NCC - 1))
                if hi == 0:
                    nc.vector.tensor_copy(fin[ot][:, n0:n0 + nsz], p[:])
                else:
                    nc.scalar.copy(fin[ot][:, n0:n0 + nsz], p[:])
            nc.sync.dma_start(d["out"][b, ot * PD:(ot + 1) * PD, :], fin[ot][:])

    # interleave batch 1's PE-dense conv work into batch 0's head phase so the
    # tensor engine stays busy (and the HAM clock stays warm) throughout.
    emit_loads(0)
    emit_q(0)
    emit_loads(1)
    emit_k(0)
    emit_vl_vt(0)
    emit_cross(0)
    emit_pair(0, 0)
    emit_q(1)
    emit_pair(0, 1)
    emit_k(1)
    emit_pair(0, 2)
    emit_pair(0, 3)
    emit_vl_vt(1)
    emit_cross(1)
    emit_pair(1, 0)
    emit_final(0, [0])
    emit_pair(1, 1)
    emit_final(0, [1])
    emit_pair(1, 2, heartbeat=True)
    emit_final(0, [2])
    emit_pair(1, 3, heartbeat=True)
    emit_final(0, [3])
    emit_final(1)


_CACHE = {}


def _build():
    if "nc" in _CACHE:
        return _CACHE["nc"], _CACHE["out"]
    nc = bacc.Bacc("TRN2", target_bir_lowering=False, debug=False,
                   num_devices=NCORES)
    d = {
        "x": nc.dram_tensor("x", [BPC, C, HW], BF16, kind="ExternalInput").ap(),
        "crossT": nc.dram_tensor("crossT", [BPC, HW, NH + 1], BF16,
                                 kind="ExternalInput").ap(),
        "tvec": nc.dram_tensor("tvec", [BPC, C, 1], F32, kind="ExternalInput").ap(),
        "out": nc.dram_tensor("out", [BPC, C, HW], BF16,
                              kind="ExternalOutput").ap(),
    }
    for wn in ("WqT", "WkT", "WvT", "Wm1T", "WrT"):
        d[wn] = nc.dram_tensor(wn, [C, C], BF16, kind="ExternalInput").ap()
    with tile.TileContext(nc) as tc:
        with ExitStack() as ctx:
            _body(ctx, tc, d)
    nc.compile()
    _CACHE["nc"], _CACHE["out"] = nc, d["out"].tensor.name
    return nc, _CACHE["out"]


def _prep_inputs(x, t, Wk, Wq, Wt_w, Wt_b, Wm, Wv, Wr_w, Wr_b):
    f = np.float32
    x = np.asarray(x, f).reshape(B, C, HW)
    t = np.asarray(t, f)
    t_m = (t @ np.asarray(Wt_w, f).T + np.asarray(Wt_b, f)).reshape(B, NH, CPH)
    # cross attention softmax on host: [B, NH, HW]
    logits = np.einsum("bhcm,bhc->bhm", x.reshape(B, NH, CPH, HW), t_m) * SCALE
    logits -= logits.max(axis=-1, keepdims=True)
    ce = np.exp(logits)
    cross = ce / ce.sum(axis=-1, keepdims=True)
    crossT = np.ones((B, HW, NH + 1), f)
    crossT[:, :, 0:NH] = cross.transpose(0, 2, 1)
    tvec = (t @ np.asarray(Wm, f)[:, C:].T).reshape(B, C, 1)
    bf = ml_dtypes.bfloat16
    com = {
        "WqT": np.ascontiguousarray(np.asarray(Wq, f).T).astype(bf),
        "WkT": np.ascontiguousarray(np.asarray(Wk, f).T).astype(bf),
        "WvT": np.ascontiguousarray(np.asarray(Wv, f).T).astype(bf),
        "Wm1T": np.ascontiguousarray(np.asarray(Wm, f)[:, :C].T).astype(bf),
        "WrT": np.ascontiguousarray(np.asarray(Wr_w, f).T).astype(bf),
    }
    maps = []
    for c in range(NCORES):
        sl = slice(c * BPC, (c + 1) * BPC)
        m = dict(com)
        m["x"] = np.ascontiguousarray(x[sl]).astype(bf)
        m["crossT"] = np.ascontiguousarray(crossT[sl]).astype(bf)
        m["tvec"] = np.ascontiguousarray(tvec[sl])
        maps.append(m)
    return maps


def kernel(x, t, Wk, Wq, Wt_w, Wt_b, Wm, Wv, Wr_w, Wr_b, _trace=False):
    nc, out_name = _build()
    maps = _prep_inputs(x, t, Wk, Wq, Wt_w, Wt_b, Wm, Wv, Wr_w, Wr_b)
    res = run_bass_kernel_spmd(nc, maps, core_ids=list(range(NCORES)),
                               trace=_trace)
    out = np.concatenate([res.results[c][out_name] for c in range(NCORES)],
                         axis=0).astype(np.float32)
    out = out.reshape(B, C, 24, 24) + np.asarray(Wr_b, np.float32).reshape(
        1, C, 1, 1)
    if _trace:
        kernel.last_results = res
    return out
